# revision 23
# baseline (speedup 1.0000x reference)
"""Trainium2 Bass kernel for HadamardTernaryLinear.

y = reshape( (FHT_g(x*alpha) @grouped w_q) -> FHT_h -> *beta ), with
w_q = BitNet-style absmean ternary quantization of weight.

Strategy: data-parallel over the 8192 tokens across 8 NeuronCores (1024
tokens/core, no collectives). Host pre-transposes x to feature-major
[D, TOKC] (i-major feature order), so all device DMAs are plain
contiguous loads — no xbar transposes. Per core, a 5-pass TensorEngine
pipeline in bf16 per 512-token supertile:

  P1 flip+FHTg : MM(lhsT=x chunk, rhs=kron-H)  -> token-major [tok,(h,i')]
                 drain scatters to tm cols (h,k,i') at 32B-run granularity
  P2 transpose : PE transpose-mode (bf16 PSUM!) -> group-major xg_h [i,tok]
                 drains read bf16 PSUM at DVE 2x
  P3 flip+GM   : MM(lhsT=xg_h chunk, rhs=wqT_h) -> token-major [tok,o]
                 drain scatters to tm3 cols (m,h,o')
  P4 transpose : PE transpose-mode (bf16)       -> A-layout ypa_m [(h,o'),tok]
  P5 mov-FHTh  : MM(lhsT=kron-H', rhs=ypa_m)    -> [(g,o''),tok] f32 PSUM
                 ACT drain applies beta*quantscale/32, stores bf16.

All drains are full-rate: scatter gathers happen on the PSUM read side
(8-byte PSUM cachelines make reordered reads free) and SBUF writes land
in 32B contiguous runs.
"""

import functools
import math
import sys

for _p in ("/opt/trn_rl_repo",):
    if _p not in sys.path:
        sys.path.insert(0, _p)

import ml_dtypes
import numpy as np

import concourse.mybir as mybir
import concourse.tile as tile
from concourse import bacc
from concourse.bass_utils import run_bass_kernel_spmd

G = 32
IO = 128  # in_o
OO = 128  # out_o
D = G * IO  # 4096
NCORES = 8
B, T = 4, 2048
BT = B * T
TOKC = BT // NCORES  # tokens per core
ST = 512  # supertile tokens
NST = TOKC // ST
CH = ST // 128  # 128-token chunks per supertile

DTB = mybir.dt.bfloat16
DTF = mybir.dt.float32
BF16 = ml_dtypes.bfloat16


def _hadamard(n):
    H = np.array([[1.0]], dtype=np.float32)
    while H.shape[0] < n:
        H = np.block([[H, H], [H, -H]])
    return H  # +-1, symmetric


def build_body(nc, tc, xin, hm2, hm3, idm, wqm, bc, yout, loop_r=1):
    """Emit the per-core program. All APs are DRAM tensors."""
    with (
        tc.tile_pool(name="const", bufs=1) as cpool,
        tc.tile_pool(name="xk", bufs=1) as xpool,
        tc.tile_pool(name="tm", bufs=4) as tmpool,
        tc.tile_pool(name="xg", bufs=1) as xgpool,
        tc.tile_pool(name="tm3", bufs=4) as tm3pool,
        tc.tile_pool(name="ypa", bufs=1) as ypapool,
        tc.tile_pool(name="yo", bufs=4) as yopool,
        tc.tile_pool(name="psf", bufs=6, space="PSUM") as psfpool,
        tc.tile_pool(name="psb", bufs=2, space="PSUM") as psbpool,
    ):
        hm2t = cpool.tile([128, 128], DTB, tag="hm2")
        nc.sync.dma_start(hm2t[:], hm2[:])
        hm3t = cpool.tile([128, 128], DTB, tag="hm3")
        nc.sync.dma_start(hm3t[:], hm3[:])
        idt = cpool.tile([128, 128], DTB, tag="id")
        nc.sync.dma_start(idt[:], idm[:])
        wqt = cpool.tile([128, G * OO], DTB, tag="wq")
        nc.sync.dma_start(wqt[:], wqm[:])
        bct = cpool.tile([128, G], DTF, tag="bc")
        nc.sync.dma_start(bct[:], bc[:])

        # whole-core x: 32 fully-contiguous 256KB loads
        xk = []
        for k in range(32):
            xt = xpool.tile([128, TOKC], DTB, tag=f"x{k}")
            nc.sync.dma_start(xt[:], xin[k * 128 : (k + 1) * 128, :])
            xk.append(xt)

        def drain_copy(i, dst, src):
            # split scatter drains 50/50 across DVE and ACT
            if i % 2 == 0:
                nc.vector.tensor_copy(dst, src)
            else:
                nc.scalar.copy(dst, src)

        def supertile(st):
            t0 = st * ST

            # ---- P1: flip+FHT_g -> tm_c [tok, (h,k,i')] bf16
            tms = []
            for c in range(CH):
                tm = tmpool.tile([128, D], DTB, tag="tm")
                tmv = tm.rearrange("p (h k i) -> p h k i", h=32, k=32)
                for kq in range(8):
                    ps = psfpool.tile([128, 512], DTF, tag="ps")
                    for kk in range(4):
                        k = kq * 4 + kk
                        nc.tensor.matmul(
                            ps[:, kk * 128 : (kk + 1) * 128],
                            lhsT=xk[k][:, t0 + c * 128 : t0 + (c + 1) * 128],
                            rhs=hm2t[:],
                            start=True,
                            stop=True,
                        )
                    src = ps.rearrange("p (kk h i) -> p h kk i", kk=4, h=32)
                    dst = tmv[:, :, 4 * kq : 4 * kq + 4, :]
                    drain_copy(c * 8 + kq, dst, src)
                tms.append(tm)

            # ---- P2: PE transpose (bf16 PSUM) -> xg [i, h, tok]
            xg = xgpool.tile([128, G, ST], DTB, tag="xg")
            for q in range(16):
                psb = psbpool.tile([128, 1024], DTB, tag="psb")
                for hh in range(2):
                    h = 2 * q + hh
                    for c in range(CH):
                        nc.tensor.transpose(
                            psb[:, hh * 512 + c * 128 : hh * 512 + (c + 1) * 128],
                            tms[c][:, h * 128 : (h + 1) * 128],
                            idt[:],
                        )
                nc.vector.tensor_copy(
                    xg[:, 2 * q : 2 * q + 2, :],
                    psb.rearrange("p (a t) -> p a t", a=2),
                )

            # ---- P3: flip+grouped MM -> tm3_c [tok, (m,h,o')] bf16
            tm3s = []
            for c in range(CH):
                tm3 = tm3pool.tile([128, D], DTB, tag="tm3")
                tm3v = tm3.rearrange("p (m h o) -> p m h o", m=32, h=32)
                for hq in range(8):
                    ps = psfpool.tile([128, 512], DTF, tag="ps")
                    for hh in range(4):
                        h = hq * 4 + hh
                        nc.tensor.matmul(
                            ps[:, hh * 128 : (hh + 1) * 128],
                            lhsT=xg[:, h, c * 128 : (c + 1) * 128],
                            rhs=wqt[:, h * 128 : (h + 1) * 128],
                            start=True,
                            stop=True,
                        )
                    src = ps.rearrange("p (hh m o) -> p m hh o", hh=4, m=32)
                    dst = tm3v[:, :, 4 * hq : 4 * hq + 4, :]
                    drain_copy(c * 8 + hq + 1, dst, src)
                tm3s.append(tm3)

            # ---- P4: PE transpose (bf16) -> ypa [(h,o'), m, tok]
            ypa = ypapool.tile([128, G, ST], DTB, tag="ypa")
            for q in range(16):
                psb = psbpool.tile([128, 1024], DTB, tag="psb")
                for mm in range(2):
                    m = 2 * q + mm
                    for c in range(CH):
                        nc.tensor.transpose(
                            psb[:, mm * 512 + c * 128 : mm * 512 + (c + 1) * 128],
                            tm3s[c][:, m * 128 : (m + 1) * 128],
                            idt[:],
                        )
                nc.vector.tensor_copy(
                    ypa[:, 2 * q : 2 * q + 2, :],
                    psb.rearrange("p (a t) -> p a t", a=2),
                )

            # ---- P5: moving FHT_h -> [(g,o''), tok] f32; ACT drain w/ beta scale
            for m in range(32):
                ps = psfpool.tile([128, 512], DTF, tag="ps")
                nc.tensor.matmul(ps[:], lhsT=hm3t[:], rhs=ypa[:, m, :], start=True, stop=True)
                yo = yopool.tile([128, ST], DTB, tag="yo")
                nc.scalar.activation(
                    yo[:],
                    ps[:],
                    mybir.ActivationFunctionType.Copy,
                    scale=bct[:, m : m + 1],
                )
                nc.sync.dma_start(yout[m * 128 : (m + 1) * 128, t0 : t0 + ST], yo[:])

        if loop_r == 1:
            for st in range(NST):
                supertile(st)
        else:
            with tc.For_i(0, loop_r, 1):
                for st in range(NST):
                    supertile(st)


@functools.lru_cache(maxsize=4)
def build_program(loop_r=1):
    nc = bacc.Bacc("TRN2", target_bir_lowering=False, debug=False)
    xin = nc.dram_tensor("xin", [D, TOKC], DTB, kind="ExternalInput").ap()
    hm2 = nc.dram_tensor("hmat2", [128, 128], DTB, kind="ExternalInput").ap()
    hm3 = nc.dram_tensor("hmat3", [128, 128], DTB, kind="ExternalInput").ap()
    idm = nc.dram_tensor("ident", [128, 128], DTB, kind="ExternalInput").ap()
    wqm = nc.dram_tensor("wqm", [128, G * OO], DTB, kind="ExternalInput").ap()
    bc = nc.dram_tensor("betacol", [128, G], DTF, kind="ExternalInput").ap()
    yout = nc.dram_tensor("yout", [D, TOKC], DTB, kind="ExternalOutput").ap()
    with tile.TileContext(nc) as tc:
        build_body(nc, tc, xin, hm2, hm3, idm, wqm, bc, yout, loop_r=loop_r)
    nc.compile()
    return nc


def host_prep(x, weight, alpha, beta):
    """Pure f32 numpy glue + bf16 casts. Returns per-core input maps."""
    H = _hadamard(G)  # [g,h] +-1, symmetric

    w = np.asarray(weight, dtype=np.float32)
    scale = np.float32(np.mean(np.abs(w))) + np.float32(1e-8)
    wq3 = np.clip(np.round(w / scale), -1.0, 1.0).astype(np.float32)  # [h,o,i]

    # x * alpha, feature-major [D, BT] with row r = i*32 + g (i-major)
    xp = np.asarray(x, dtype=np.float32).reshape(BT, G, IO) * np.asarray(
        alpha, dtype=np.float32
    )[None]
    xin_fm = np.ascontiguousarray(xp.transpose(2, 1, 0)).reshape(D, BT).astype(BF16)

    # hmat2[(i',g),(h,i'')] = H[g,h] * (i'==i'')
    hmat2 = (
        np.eye(4, dtype=np.float32)[:, None, None, :] * H[None, :, :, None]
    ).reshape(128, 128).astype(BF16)
    # hmat3[(h,o'),(g,o'')] = H[h,g] * (o'==o'')
    hmat3 = np.kron(H, np.eye(4, dtype=np.float32)).astype(BF16)
    ident = np.eye(128, dtype=np.float32).astype(BF16)
    # wqm[i, (h,o)] = wq3[h,o,i]
    wq_sb = np.ascontiguousarray(wq3.transpose(2, 0, 1)).reshape(IO, G * OO).astype(BF16)

    # betacol[(g,o''), m] = beta[g, 4m+o''] * scale / 32
    beta_f = np.asarray(beta, dtype=np.float32) * (scale / np.float32(G))  # [g,o]
    bc = np.ascontiguousarray(
        beta_f.reshape(G, G, 4).transpose(0, 2, 1)
    ).reshape(128, G).astype(np.float32)

    in_maps = []
    for c in range(NCORES):
        in_maps.append(
            {
                "xin": np.ascontiguousarray(xin_fm[:, c * TOKC : (c + 1) * TOKC]),
                "hmat2": hmat2,
                "hmat3": hmat3,
                "ident": ident,
                "wqm": wq_sb,
                "betacol": bc,
            }
        )
    return in_maps


def host_post(results):
    ydev = np.stack([np.asarray(r["yout"], dtype=np.float32) for r in results])
    # row r = m*128 + g*4 + o''  ->  feature (g, o = 4m+o''); want y[tok, g*128+o]
    y = ydev.reshape(NCORES, 32, 32, 4, TOKC)  # [c, m, g, o'', t]
    y = y.transpose(0, 4, 2, 1, 3)  # [c, t, g, m, o'']
    return np.ascontiguousarray(y).reshape(B, T, D)


def kernel(x, weight, alpha, beta):
    nc = build_program(loop_r=1)
    in_maps = host_prep(x, weight, alpha, beta)
    res = run_bass_kernel_spmd(nc, in_maps, core_ids=list(range(NCORES)))
    return host_post(res.results)
